# revision 13
# baseline (speedup 1.0000x reference)
"""Causal self-attention (B=4, S=2048, Dm=1024, H=16, Dh=64) on 8 trn2 NeuronCores.

Sharding: core c -> (batch b = c//2, head-group g = c%2 covering 8 of 16 heads).
Each core computes QKV projection for its (batch, head-slice), RoPE, causal
attention (materializing its attn slice [8, 2048, 2048]), and a partial
out-projection [2048, 1024]. Host sums the two partials per batch and
reassembles attn.
"""

import numpy as np

import concourse.mybir as mybir
from concourse import bacc
from concourse.tile import TileContext
from concourse.bass_utils import run_bass_kernel_spmd
from concourse.masks import make_identity

F32 = mybir.dt.float32
# Matmul operand dtype: float32r = full fp32 storage, reduced-precision PE
# multiply at 4x the fp32 rate. Flip to F32 for full-precision matmuls.
MM_DT = mybir.dt.float32r

B, S, DM, H, DH = 4, 2048, 1024, 16, 64
HL = H // 2          # heads per core = 8
ML = HL * DH         # local head-dim width = 512
P = 128
NQB = S // P         # 16 q-blocks
WRITE_ZEROS = True
DEBUG = False   # explicitly write zeros to the strictly-upper attn blocks

EXP_FN = mybir.ActivationFunctionType.Exp
MULT = mybir.AluOpType.mult
ADD = mybir.AluOpType.add


def _build():
    nc = bacc.Bacc(None, target_bir_lowering=False)

    xT = nc.dram_tensor("xT", [DM, S], F32, kind="ExternalInput")
    wq = nc.dram_tensor("wq", [DM, ML], F32, kind="ExternalInput")
    wk = nc.dram_tensor("wk", [DM, ML], F32, kind="ExternalInput")
    wv = nc.dram_tensor("wv", [DM, ML], F32, kind="ExternalInput")
    wo = nc.dram_tensor("wo", [ML, DM], F32, kind="ExternalInput")
    bq = nc.dram_tensor("bq", [P, 4], F32, kind="ExternalInput")
    bk = nc.dram_tensor("bk", [P, 4], F32, kind="ExternalInput")
    bv = nc.dram_tensor("bv", [P, ML], F32, kind="ExternalInput")
    bo = nc.dram_tensor("bo", [P, DM], F32, kind="ExternalInput")
    cosq = nc.dram_tensor("cosq", [P, S], F32, kind="ExternalInput")
    sinq = nc.dram_tensor("sinq", [P, S], F32, kind="ExternalInput")
    cosk = nc.dram_tensor("cosk", [P, S], F32, kind="ExternalInput")
    sink = nc.dram_tensor("sink", [P, S], F32, kind="ExternalInput")
    maskd = nc.dram_tensor("maskd", [P, P], F32, kind="ExternalInput")
    maskdT = nc.dram_tensor("maskdT", [P, P], F32, kind="ExternalInput")

    attn_o = nc.dram_tensor("attn_o", [HL, S, S], F32, kind="ExternalOutput")
    out_o = nc.dram_tensor("out_o", [S, DM], F32, kind="ExternalOutput")
    if DEBUG:
        dbg_outT = nc.dram_tensor("dbg_outT", [P, 4, S], F32, kind="ExternalOutput")
        dbg_r2 = nc.dram_tensor("dbg_r2", [P, NQB], F32, kind="ExternalOutput")
        dbg_bc = nc.dram_tensor("dbg_bc", [P, 1024], F32, kind="ExternalOutput")
        dbg_aT = nc.dram_tensor("dbg_aT", [P, 1024], F32, kind="ExternalOutput")
        dbg_psO = nc.dram_tensor("dbg_psO", [64, 1024], F32, kind="ExternalOutput")

    xT_r = xT.rearrange("(ko p) s -> p ko s", p=P)      # [128, 8, 2048]
    wq_r = wq.rearrange("(ko p) m -> p ko m", p=P)      # [128, 8, 512]
    wk_r = wk.rearrange("(ko p) m -> p ko m", p=P)
    wv_r = wv.rearrange("(ko p) m -> p ko m", p=P)
    wo_r = wo.rearrange("(mo p) n -> p mo n", p=P)      # [128, 4, 1024]

    with TileContext(nc) as tc:
        with (
            tc.tile_pool(name="persist", bufs=1) as persist,
            tc.tile_pool(name="stats", bufs=4) as stats,
        ):
            # persistent SBUF tensors
            qT_sb = persist.tile([P, 4, S], MM_DT, tag="qT")    # [m, mt, s]
            kT_sb = persist.tile([P, 4, S], MM_DT, tag="kT")
            v_sb = persist.tile([P, NQB, ML], MM_DT, tag="v")   # [s_in, s_blk, m]
            outT_sb = persist.tile([P, 4, S], MM_DT, tag="outT")
            ident = persist.tile([P, P], F32, tag="ident")
            make_identity(nc, ident[:])
            mask_sb = persist.tile([P, P], F32, tag="mask")
            nc.sync.dma_start(mask_sb[:], maskd[:])
            maskT_sb = persist.tile([P, P], F32, tag="maskT")
            nc.sync.dma_start(maskT_sb[:], maskdT[:])

            # ---------------- Phase 1a: qT / kT (+bias, RoPE) ----------------
            with (
                tc.tile_pool(name="w1", bufs=1) as w1,
                tc.tile_pool(name="xt1", bufs=1) as xt1,
                tc.tile_pool(name="tab1", bufs=1) as tab1,
                tc.tile_pool(name="tmp1", bufs=2) as tmp1,
                tc.tile_pool(name="ps1", bufs=4, space="PSUM") as ps1,
            ):
                wq_sb = w1.tile([P, 8, ML], F32, tag="wq")
                wk_sb = w1.tile([P, 8, ML], F32, tag="wk")
                bq_sb = w1.tile([P, 4], F32, tag="bq")
                bk_sb = w1.tile([P, 4], F32, tag="bk")
                nc.sync.dma_start(wq_sb[:], wq_r)
                nc.sync.dma_start(wk_sb[:], wk_r)
                nc.sync.dma_start(bq_sb[:], bq[:])
                nc.sync.dma_start(bk_sb[:], bk[:])

                for sc in range(4):
                    ssl = slice(sc * 512, (sc + 1) * 512)
                    xt = xt1.tile([P, 8, 512], F32, tag="xt")
                    nc.sync.dma_start(xt[:], xT_r[:, :, ssl])
                    cq = tab1.tile([P, 512], F32, tag="cq")
                    sq = tab1.tile([P, 512], F32, tag="sq")
                    ck = tab1.tile([P, 512], F32, tag="ck")
                    sk = tab1.tile([P, 512], F32, tag="sk")
                    nc.sync.dma_start(cq[:], cosq[:, ssl])
                    nc.sync.dma_start(sq[:], sinq[:, ssl])
                    nc.sync.dma_start(ck[:], cosk[:, ssl])
                    nc.sync.dma_start(sk[:], sink[:, ssl])

                    for mt in range(4):
                        msl = slice(mt * P, (mt + 1) * P)
                        for (w_sb, b_sb, ct, st_, dst) in (
                            (wq_sb, bq_sb, cq, sq, qT_sb),
                            (wk_sb, bk_sb, ck, sk, kT_sb),
                        ):
                            ps = ps1.tile([P, 512], F32, tag="ps")
                            for kt in range(8):
                                nc.tensor.matmul(
                                    ps[:], w_sb[:, kt, msl], xt[:, kt],
                                    start=(kt == 0), stop=(kt == 7),
                                )
                            qf = tmp1.tile([P, 512], F32, tag="qf")
                            nc.vector.tensor_scalar(
                                qf[:], ps[:], b_sb[:, mt:mt + 1], None, ADD
                            )
                            # rotate-half: rot = [-q[32:64], q[0:32]] per 64-head
                            rot = tmp1.tile([P, 512], F32, tag="rot")
                            nc.gpsimd.tensor_scalar_mul(rot[0:32], qf[32:64], -1.0)
                            nc.gpsimd.tensor_copy(rot[32:64], qf[0:32])
                            nc.gpsimd.tensor_scalar_mul(rot[64:96], qf[96:128], -1.0)
                            nc.gpsimd.tensor_copy(rot[96:128], qf[64:96])
                            t1 = tmp1.tile([P, 512], F32, tag="t1")
                            t2 = tmp1.tile([P, 512], F32, tag="t2")
                            nc.vector.tensor_tensor(t1[:], qf[:], ct[:], MULT)
                            nc.gpsimd.tensor_tensor(t2[:], rot[:], st_[:], MULT)
                            nc.vector.tensor_tensor(
                                dst[:, mt, ssl], t1[:], t2[:], ADD
                            )

            # ---------------- Phase 1b: v ----------------
            with (
                tc.tile_pool(name="w2", bufs=1) as w2,
                tc.tile_pool(name="xt2", bufs=2) as xt2,
                tc.tile_pool(name="ps2p", bufs=4, space="PSUM") as ps2p,
            ):
                wv_sb = w2.tile([P, 8, ML], F32, tag="wv")
                bv_sb = w2.tile([P, ML], F32, tag="bv")
                nc.sync.dma_start(wv_sb[:], wv_r)
                nc.sync.dma_start(bv_sb[:], bv[:])
                for sc in range(4):
                    xt = xt2.tile([P, 8, 512], F32, tag="xt")
                    nc.sync.dma_start(xt[:], xT_r[:, :, sc * 512:(sc + 1) * 512])
                    for sbk in range(4):
                        ps = ps2p.tile([P, ML], F32, tag="psv")
                        for kt in range(8):
                            nc.tensor.matmul(
                                ps[:], xt[:, kt, sbk * P:(sbk + 1) * P],
                                wv_sb[:, kt],
                                start=(kt == 0), stop=(kt == 7),
                            )
                        nc.vector.tensor_tensor(
                            v_sb[:, sc * 4 + sbk, :], ps[:], bv_sb[:], ADD
                        )

            # ---------------- Phase 2: attention per head-pair ----------------
            with tc.tile_pool(name="r2pool", bufs=1) as r2pool:
                if WRITE_ZEROS:
                    zero_sb = r2pool.tile([P, 1024], F32, tag="zero", name="zero")
                    nc.vector.memset(zero_sb[:], 0.0)
                for hp in range(4):
                    r2 = {
                        0: r2pool.tile([P, NQB], F32, tag="r2A", name="r2A"),
                        1: r2pool.tile([P, NQB], F32, tag="r2B", name="r2B"),
                    }
                    # --- 2a: scores [i, j], softmax, attn out, rsum2 ---
                    with (
                        tc.tile_pool(name="strip", bufs=1, space="PSUM") as strip,
                        tc.tile_pool(name="attp", bufs=4) as attp,
                    ):
                        for qb in range(NQB):
                            W = (qb + 1) * P
                            qsl = slice(qb * P, (qb + 1) * P)
                            pstrip = {
                                0: strip.tile([P, S], F32, tag="sA", name="sA"),
                                1: strip.tile([P, S], F32, tag="sB", name="sB"),
                            }
                            for h2 in (0, 1):
                                base = h2 * 64
                                for cc in range((W + 511) // 512):
                                    N = min(512, W - cc * 512)
                                    nc.tensor.matmul(
                                        pstrip[h2][:, cc * 512:cc * 512 + N],
                                        qT_sb[base:base + 64, hp, qsl],
                                        kT_sb[base:base + 64, hp,
                                              cc * 512:cc * 512 + N],
                                        start=True, stop=True,
                                        tile_position=(base, 0),
                                    )
                            for h2 in (0, 1):
                                ps_ = pstrip[h2]
                                nc.vector.tensor_tensor(
                                    ps_[:, W - P:W], ps_[:, W - P:W],
                                    mask_sb[:], ADD,
                                )
                                negmax = stats.tile([P, 1], F32, tag="nm")
                                nc.vector.reduce_max(
                                    negmax[:], ps_[:, :W],
                                    axis=mybir.AxisListType.X, negate=True,
                                )
                                at = attp.tile([P, S], F32, tag="at")
                                ssum = stats.tile([P, 1], F32, tag="ss")
                                nc.scalar.activation(
                                    at[:, :W], ps_[:, :W], EXP_FN,
                                    bias=negmax[:], scale=1.0,
                                    accum_out=ssum[:],
                                )
                                rsum = stats.tile([P, 1], F32, tag="rs")
                                nc.vector.reciprocal(rsum[:], ssum[:])
                                emax = stats.tile([P, 1], F32, tag="em")
                                nc.scalar.activation(emax[:], negmax[:], EXP_FN)
                                nc.vector.tensor_tensor(
                                    r2[h2][:, qb:qb + 1], rsum[:], emax[:], MULT
                                )
                                nc.vector.tensor_scalar_mul(
                                    at[:, :W], at[:, :W], rsum[:]
                                )
                                h_ = hp * 2 + h2
                                nc.sync.dma_start(
                                    attn_o[h_, qsl, 0:W], at[:, :W]
                                )
                                if WRITE_ZEROS and W < S:
                                    zw = W
                                    while zw < S:
                                        zn = min(1024, S - zw)
                                        nc.sync.dma_start(
                                            attn_o[h_, qsl, zw:zw + zn],
                                            zero_sb[:, :zn],
                                        )
                                        zw += zn
                    if DEBUG and hp == 0:
                        nc.sync.dma_start(dbg_r2[:], r2[0][:])
                    # --- 2b: scoresT -> exp -> attn@v -> outT (per head) ---
                    for h2 in (0, 1):
                        base = h2 * 64
                        with (
                            tc.tile_pool(name="ps2b", bufs=2, space="PSUM") as ps2b,
                            tc.tile_pool(name="psO", bufs=1, space="PSUM") as psOp,
                            tc.tile_pool(name="ptr", bufs=2, space="PSUM") as ptrp,
                            tc.tile_pool(name="atT", bufs=3) as atT,
                            tc.tile_pool(name="bcp", bufs=2) as bcp,
                        ):
                            for c2 in range(2):
                                jmax = 8 * (c2 + 1)
                                isl0 = c2 * 1024
                                psO = psOp.tile([64, 1024], F32, tag="psO")
                                for jt in range(jmax):
                                    jsl = slice(jt * P, (jt + 1) * P)
                                    jd_lo = jt * P - isl0  # diag offset in chunk
                                    ih0 = max(0, jd_lo) // 512
                                    pre = max(0, jd_lo) - ih0 * 512
                                    ps2 = ps2b.tile([P, 1024], F32, tag="ps2")
                                    for ih in range(ih0, 2):
                                        nc.tensor.matmul(
                                            ps2[:, ih * 512:(ih + 1) * 512],
                                            kT_sb[base:base + 64, hp, jsl],
                                            qT_sb[base:base + 64, hp,
                                                  isl0 + ih * 512:
                                                  isl0 + (ih + 1) * 512],
                                            start=True, stop=True,
                                            tile_position=(base, 0),
                                        )
                                    if jd_lo >= 0:
                                        # mask j > i on the diagonal block
                                        nc.vector.tensor_tensor(
                                            ps2[:, jd_lo:jd_lo + P],
                                            ps2[:, jd_lo:jd_lo + P],
                                            maskT_sb[:], ADD,
                                        )
                                    aT = atT.tile([P, 1024], MM_DT, tag="aT")
                                    if pre > 0:
                                        nc.vector.memset(
                                            aT[:, ih0 * 512:ih0 * 512 + pre]
                                            .bitcast(F32),
                                            0.0,
                                        )
                                    nc.scalar.activation(
                                        aT[:, ih0 * 512 + pre:1024],
                                        ps2[:, ih0 * 512 + pre:1024], EXP_FN,
                                    )
                                    if DEBUG and hp == 0 and h2 == 0 and c2 == 0 and jt == 0:
                                        nc.sync.dma_start(dbg_aT[:], aT[:].bitcast(F32))
                                    vbase = (2 * hp + h2) * 64
                                    for ih in range(ih0, 2):
                                        last_jt = min(
                                            jmax - 1,
                                            (isl0 + (ih + 1) * 512) // P - 1,
                                        )
                                        nc.tensor.matmul(
                                            psO[:, ih * 512:(ih + 1) * 512],
                                            v_sb[:, jt, vbase:vbase + 64],
                                            aT[:, ih * 512:(ih + 1) * 512],
                                            start=(jt == 0), stop=(jt == last_jt),
                                        )
                                # broadcast rsum2 for these 8 q-blocks to [128,1024]
                                bc = bcp.tile([P, 1024], F32, tag="bc")
                                for qq in range(8):
                                    qbi = c2 * 8 + qq
                                    pt = ptrp.tile([1, P], F32, tag="pt")
                                    nc.tensor.transpose(
                                        pt[:], r2[h2][:, qbi:qbi + 1], ident[:]
                                    )
                                    row = stats.tile([1, P], F32, tag="row")
                                    nc.vector.tensor_copy(row[:], pt[:])
                                    nc.gpsimd.partition_broadcast(
                                        bc[:, qq * P:(qq + 1) * P], row[:]
                                    )
                                nc.vector.tensor_tensor(
                                    outT_sb[base:base + 64, hp,
                                            isl0:isl0 + 1024],
                                    psO[:], bc[0:64, :], MULT,
                                )
                                if DEBUG and hp == 0 and h2 == 0 and c2 == 0:
                                    nc.sync.dma_start(dbg_bc[:], bc[:])
                                    dps = bcp.tile([64, 1024], F32, tag="dps", name="dps")
                                    nc.vector.tensor_copy(dps[:], psO[:])
                                    nc.sync.dma_start(dbg_psO[:], dps[:])

            if DEBUG:
                with tc.tile_pool(name="dbgp", bufs=2) as dbgp:
                    for mt in range(4):
                        dt_ = dbgp.tile([P, S], F32, tag="dt")
                        nc.vector.tensor_copy(dt_[:], outT_sb[:, mt, :].bitcast(F32))
                        nc.sync.dma_start(dbg_outT[:, mt, :], dt_[:])
            # ---------------- Phase 3: out projection ----------------
            with (
                tc.tile_pool(name="w3", bufs=1) as w3,
                tc.tile_pool(name="ps3", bufs=4, space="PSUM") as ps3,
                tc.tile_pool(name="o3", bufs=3) as o3,
            ):
                wo_sb = w3.tile([P, 4, DM], MM_DT, tag="wo")
                nc.gpsimd.dma_start(wo_sb[:], wo_r)
                bo_sb = w3.tile([P, DM], F32, tag="bo")
                nc.sync.dma_start(bo_sb[:], bo[:])
                for st in range(NQB):
                    ssl = slice(st * P, (st + 1) * P)
                    for ncn in range(2):
                        nsl = slice(ncn * 512, (ncn + 1) * 512)
                        ps = ps3.tile([P, 512], F32, tag="ps")
                        for mt in range(4):
                            nc.tensor.matmul(
                                ps[:], outT_sb[:, mt, ssl], wo_sb[:, mt, nsl],
                                start=(mt == 0), stop=(mt == 3),
                            )
                        ot = o3.tile([P, 512], F32, tag="ot")
                        nc.vector.tensor_tensor(ot[:], ps[:], bo_sb[:, nsl], ADD)
                        nc.sync.dma_start(out_o[ssl, nsl], ot[:])

    nc.compile()
    return nc


_NC_CACHE = None
_last_in_maps = None


def _get_nc():
    global _NC_CACHE
    if _NC_CACHE is None:
        _NC_CACHE = _build()
    return _NC_CACHE


def _rope_tables():
    # mirror reference.apply_rope in float32
    inv_freq = (
        1.0 / (np.float32(10000.0)
               ** (np.arange(0, DH, 2, dtype=np.float32) / np.float32(DH)))
    ).astype(np.float32)
    t = np.arange(S, dtype=np.float32)
    freqs = t[:, None] * inv_freq[None, :]                    # [S, 32]
    cos = np.concatenate([np.cos(freqs), np.cos(freqs)], axis=-1)  # [S, 64]
    sin = np.concatenate([np.sin(freqs), np.sin(freqs)], axis=-1)
    cosT = np.ascontiguousarray(cos.T)                        # [64, S]
    sinT = np.ascontiguousarray(sin.T)
    cos2 = np.tile(cosT, (2, 1)).astype(np.float32)           # [128, S]
    sin2 = np.tile(sinT, (2, 1)).astype(np.float32)
    return cos2, sin2


def kernel(x, w_qkv, b_qkv, w_out, b_out):
    x = np.asarray(x, dtype=np.float32)
    w_qkv = np.asarray(w_qkv, dtype=np.float32)
    b_qkv = np.asarray(b_qkv, dtype=np.float32)
    w_out = np.asarray(w_out, dtype=np.float32)
    b_out = np.asarray(b_out, dtype=np.float32)

    nc = _get_nc()
    cos2, sin2 = _rope_tables()
    scale = np.float32(1.0 / np.sqrt(DH))  # 1/8, exact power of two
    cosq = cos2 * scale
    sinq = sin2 * scale
    mask = np.triu(np.full((P, P), -1e30, dtype=np.float32), 1)

    in_maps = []
    for c in range(8):
        b, g = c // 2, c % 2
        h0 = g * HL
        col = slice(h0 * DH, h0 * DH + ML)
        wq_s = np.ascontiguousarray(w_qkv[:, 0 * DM:1 * DM][:, col])
        wk_s = np.ascontiguousarray(w_qkv[:, 1 * DM:2 * DM][:, col])
        wv_s = np.ascontiguousarray(w_qkv[:, 2 * DM:3 * DM][:, col])
        bq_s = b_qkv[0 * DM:1 * DM][col].reshape(4, P).T
        bk_s = b_qkv[1 * DM:2 * DM][col].reshape(4, P).T
        bv_s = np.broadcast_to(b_qkv[2 * DM:3 * DM][col], (P, ML))
        wo_s = np.ascontiguousarray(w_out[g * ML:(g + 1) * ML, :])
        bo_eff = b_out if g == 0 else np.zeros_like(b_out)
        bo_s = np.broadcast_to(bo_eff, (P, DM))
        in_maps.append({
            "xT": np.ascontiguousarray(x[b].T),
            "wq": wq_s, "wk": wk_s, "wv": wv_s, "wo": wo_s,
            "bq": np.ascontiguousarray(bq_s),
            "bk": np.ascontiguousarray(bk_s),
            "bv": np.ascontiguousarray(bv_s),
            "bo": np.ascontiguousarray(bo_s),
            "cosq": cosq, "sinq": sinq, "cosk": cos2, "sink": sin2,
            "maskd": mask, "maskdT": np.ascontiguousarray(mask.T),
        })

    global _last_in_maps
    _last_in_maps = in_maps
    res = run_bass_kernel_spmd(nc, in_maps, core_ids=list(range(8)))

    out = np.empty((B, S, DM), dtype=np.float32)
    attn = np.empty((B, H, S, S), dtype=np.float32)
    for c in range(8):
        b, g = c // 2, c % 2
        r = res.results[c]
        attn[b, g * HL:(g + 1) * HL] = r["attn_o"]
        if g == 0:
            out[b] = r["out_o"]
        else:
            out[b] += r["out_o"]
    return out, attn


# revision 14
# speedup vs baseline: 1.3801x; 1.3801x over previous
"""Causal self-attention (B=4, S=2048, Dm=1024, H=16, Dh=64) on 8 trn2 NeuronCores.

Sharding: core c -> (batch b = c//2, head-group g = c%2 covering 8 of 16 heads).
Each core computes QKV projection for its (batch, head-slice), RoPE, causal
attention (materializing its attn slice [8, 2048, 2048]), and a partial
out-projection [2048, 1024]. Host sums the two partials per batch and
reassembles attn.
"""

import numpy as np

import concourse.mybir as mybir
from concourse import bacc
from concourse.tile import TileContext
from concourse.bass_utils import run_bass_kernel_spmd
from concourse.masks import make_identity

F32 = mybir.dt.float32
# Matmul operand dtype: float32r = full fp32 storage, reduced-precision PE
# multiply at 4x the fp32 rate. Flip to F32 for full-precision matmuls.
MM_DT = mybir.dt.float32r

B, S, DM, H, DH = 4, 2048, 1024, 16, 64
HL = H // 2          # heads per core = 8
ML = HL * DH         # local head-dim width = 512
P = 128
NQB = S // P         # 16 q-blocks
WRITE_ZEROS = False
DEBUG = False   # explicitly write zeros to the strictly-upper attn blocks

EXP_FN = mybir.ActivationFunctionType.Exp
MULT = mybir.AluOpType.mult
ADD = mybir.AluOpType.add


def _build():
    nc = bacc.Bacc(None, target_bir_lowering=False)

    xT = nc.dram_tensor("xT", [DM, S], F32, kind="ExternalInput")
    wq = nc.dram_tensor("wq", [DM, ML], F32, kind="ExternalInput")
    wk = nc.dram_tensor("wk", [DM, ML], F32, kind="ExternalInput")
    wv = nc.dram_tensor("wv", [DM, ML], F32, kind="ExternalInput")
    wo = nc.dram_tensor("wo", [ML, DM], F32, kind="ExternalInput")
    bq = nc.dram_tensor("bq", [P, 4], F32, kind="ExternalInput")
    bk = nc.dram_tensor("bk", [P, 4], F32, kind="ExternalInput")
    bv = nc.dram_tensor("bv", [P, ML], F32, kind="ExternalInput")
    bo = nc.dram_tensor("bo", [P, DM], F32, kind="ExternalInput")
    cosq = nc.dram_tensor("cosq", [P, S], F32, kind="ExternalInput")
    sinq = nc.dram_tensor("sinq", [P, S], F32, kind="ExternalInput")
    cosk = nc.dram_tensor("cosk", [P, S], F32, kind="ExternalInput")
    sink = nc.dram_tensor("sink", [P, S], F32, kind="ExternalInput")
    maskd = nc.dram_tensor("maskd", [P, P], F32, kind="ExternalInput")
    maskdT = nc.dram_tensor("maskdT", [P, P], F32, kind="ExternalInput")

    attn_o = nc.dram_tensor("attn_o", [HL, S, S], F32, kind="ExternalOutput")
    out_o = nc.dram_tensor("out_o", [S, DM], F32, kind="ExternalOutput")
    if DEBUG:
        dbg_outT = nc.dram_tensor("dbg_outT", [P, 4, S], F32, kind="ExternalOutput")
        dbg_r2 = nc.dram_tensor("dbg_r2", [P, NQB], F32, kind="ExternalOutput")
        dbg_bc = nc.dram_tensor("dbg_bc", [P, 1024], F32, kind="ExternalOutput")
        dbg_aT = nc.dram_tensor("dbg_aT", [P, 1024], F32, kind="ExternalOutput")
        dbg_psO = nc.dram_tensor("dbg_psO", [64, 1024], F32, kind="ExternalOutput")

    xT_r = xT.rearrange("(ko p) s -> p ko s", p=P)      # [128, 8, 2048]
    wq_r = wq.rearrange("(ko p) m -> p ko m", p=P)      # [128, 8, 512]
    wk_r = wk.rearrange("(ko p) m -> p ko m", p=P)
    wv_r = wv.rearrange("(ko p) m -> p ko m", p=P)
    wo_r = wo.rearrange("(mo p) n -> p mo n", p=P)      # [128, 4, 1024]

    with TileContext(nc) as tc:
        with (
            tc.tile_pool(name="persist", bufs=1) as persist,
            tc.tile_pool(name="stats", bufs=4) as stats,
        ):
            # persistent SBUF tensors
            qT_sb = persist.tile([P, 4, S], MM_DT, tag="qT")    # [m, mt, s]
            kT_sb = persist.tile([P, 4, S], MM_DT, tag="kT")
            v_sb = persist.tile([P, NQB, ML], MM_DT, tag="v")   # [s_in, s_blk, m]
            outT_sb = persist.tile([P, 4, S], MM_DT, tag="outT")
            ident = persist.tile([P, P], F32, tag="ident")
            make_identity(nc, ident[:])
            mask_sb = persist.tile([P, P], F32, tag="mask")
            nc.sync.dma_start(mask_sb[:], maskd[:])
            maskT_sb = persist.tile([P, P], F32, tag="maskT")
            nc.sync.dma_start(maskT_sb[:], maskdT[:])

            # ---------------- Phase 1a: qT / kT (+bias, RoPE) ----------------
            with (
                tc.tile_pool(name="w1", bufs=1) as w1,
                tc.tile_pool(name="xt1", bufs=1) as xt1,
                tc.tile_pool(name="tab1", bufs=1) as tab1,
                tc.tile_pool(name="tmp1", bufs=2) as tmp1,
                tc.tile_pool(name="ps1", bufs=4, space="PSUM") as ps1,
            ):
                wq_sb = w1.tile([P, 8, ML], F32, tag="wq")
                wk_sb = w1.tile([P, 8, ML], F32, tag="wk")
                bq_sb = w1.tile([P, 4], F32, tag="bq")
                bk_sb = w1.tile([P, 4], F32, tag="bk")
                nc.sync.dma_start(wq_sb[:], wq_r)
                nc.sync.dma_start(wk_sb[:], wk_r)
                nc.sync.dma_start(bq_sb[:], bq[:])
                nc.sync.dma_start(bk_sb[:], bk[:])

                for sc in range(4):
                    ssl = slice(sc * 512, (sc + 1) * 512)
                    xt = xt1.tile([P, 8, 512], F32, tag="xt")
                    nc.sync.dma_start(xt[:], xT_r[:, :, ssl])
                    cq = tab1.tile([P, 512], F32, tag="cq")
                    sq = tab1.tile([P, 512], F32, tag="sq")
                    ck = tab1.tile([P, 512], F32, tag="ck")
                    sk = tab1.tile([P, 512], F32, tag="sk")
                    nc.sync.dma_start(cq[:], cosq[:, ssl])
                    nc.sync.dma_start(sq[:], sinq[:, ssl])
                    nc.sync.dma_start(ck[:], cosk[:, ssl])
                    nc.sync.dma_start(sk[:], sink[:, ssl])

                    for mt in range(4):
                        msl = slice(mt * P, (mt + 1) * P)
                        for (w_sb, b_sb, ct, st_, dst) in (
                            (wq_sb, bq_sb, cq, sq, qT_sb),
                            (wk_sb, bk_sb, ck, sk, kT_sb),
                        ):
                            ps = ps1.tile([P, 512], F32, tag="ps")
                            for kt in range(8):
                                nc.tensor.matmul(
                                    ps[:], w_sb[:, kt, msl], xt[:, kt],
                                    start=(kt == 0), stop=(kt == 7),
                                )
                            qf = tmp1.tile([P, 512], F32, tag="qf")
                            nc.vector.tensor_scalar(
                                qf[:], ps[:], b_sb[:, mt:mt + 1], None, ADD
                            )
                            # rotate-half: rot = [-q[32:64], q[0:32]] per 64-head
                            rot = tmp1.tile([P, 512], F32, tag="rot")
                            nc.gpsimd.tensor_scalar_mul(rot[0:32], qf[32:64], -1.0)
                            nc.gpsimd.tensor_copy(rot[32:64], qf[0:32])
                            nc.gpsimd.tensor_scalar_mul(rot[64:96], qf[96:128], -1.0)
                            nc.gpsimd.tensor_copy(rot[96:128], qf[64:96])
                            t1 = tmp1.tile([P, 512], F32, tag="t1")
                            t2 = tmp1.tile([P, 512], F32, tag="t2")
                            nc.vector.tensor_tensor(t1[:], qf[:], ct[:], MULT)
                            nc.gpsimd.tensor_tensor(t2[:], rot[:], st_[:], MULT)
                            nc.vector.tensor_tensor(
                                dst[:, mt, ssl], t1[:], t2[:], ADD
                            )

            # ---------------- Phase 1b: v ----------------
            with (
                tc.tile_pool(name="w2", bufs=1) as w2,
                tc.tile_pool(name="xt2", bufs=2) as xt2,
                tc.tile_pool(name="ps2p", bufs=4, space="PSUM") as ps2p,
            ):
                wv_sb = w2.tile([P, 8, ML], F32, tag="wv")
                bv_sb = w2.tile([P, ML], F32, tag="bv")
                nc.sync.dma_start(wv_sb[:], wv_r)
                nc.sync.dma_start(bv_sb[:], bv[:])
                for sc in range(4):
                    xt = xt2.tile([P, 8, 512], F32, tag="xt")
                    nc.sync.dma_start(xt[:], xT_r[:, :, sc * 512:(sc + 1) * 512])
                    for sbk in range(4):
                        ps = ps2p.tile([P, ML], F32, tag="psv")
                        for kt in range(8):
                            nc.tensor.matmul(
                                ps[:], xt[:, kt, sbk * P:(sbk + 1) * P],
                                wv_sb[:, kt],
                                start=(kt == 0), stop=(kt == 7),
                            )
                        nc.vector.tensor_tensor(
                            v_sb[:, sc * 4 + sbk, :], ps[:], bv_sb[:], ADD
                        )

            # ---------------- Phase 2: attention per head-pair ----------------
            with tc.tile_pool(name="r2pool", bufs=1) as r2pool:
                for hp in range(4):
                    r2 = {
                        0: r2pool.tile([P, NQB], F32, tag="r2A", name="r2A"),
                        1: r2pool.tile([P, NQB], F32, tag="r2B", name="r2B"),
                    }
                    # --- 2a: scores [i, j], softmax, attn out, rsum2 ---
                    with (
                        tc.tile_pool(name="strip", bufs=2, space="PSUM") as strip,
                        tc.tile_pool(name="attp", bufs=4) as attp,
                    ):
                        for qb in range(NQB):
                            W = (qb + 1) * P
                            qsl = slice(qb * P, (qb + 1) * P)
                            nsub = 1 if W <= 1024 else 2
                            for h2 in (0, 1):
                                base = h2 * 64
                                at = attp.tile([P, S], F32, tag="at")
                                ssums = []
                                for sub in range(nsub):
                                    ibase = sub * 1024
                                    wz = min(1024, W - ibase)
                                    ps_ = strip.tile(
                                        [P, 1024], F32,
                                        tag=f"s{h2}", name=f"s{h2}",
                                    )
                                    for cc in range((wz + 511) // 512):
                                        N = min(512, wz - cc * 512)
                                        j0 = ibase + cc * 512
                                        nc.tensor.matmul(
                                            ps_[:, cc * 512:cc * 512 + N],
                                            qT_sb[base:base + 64, hp, qsl],
                                            kT_sb[base:base + 64, hp,
                                                  j0:j0 + N],
                                            start=True, stop=True,
                                            tile_position=(base, 0),
                                        )
                                    if ibase + wz == W:
                                        # diagonal block lives in this sub-strip
                                        dlo = W - P - ibase
                                        nc.vector.tensor_tensor(
                                            ps_[:, dlo:dlo + P],
                                            ps_[:, dlo:dlo + P],
                                            mask_sb[:], ADD,
                                        )
                                    ssum = stats.tile([P, 1], F32, tag="ss")
                                    nc.scalar.activation(
                                        at[:, ibase:ibase + wz],
                                        ps_[:, :wz], EXP_FN,
                                        accum_out=ssum[:],
                                    )
                                    ssums.append(ssum)
                                if nsub == 2:
                                    stot = stats.tile([P, 1], F32, tag="st")
                                    nc.vector.tensor_tensor(
                                        stot[:], ssums[0][:], ssums[1][:], ADD
                                    )
                                else:
                                    stot = ssums[0]
                                nc.vector.reciprocal(
                                    r2[h2][:, qb:qb + 1], stot[:]
                                )
                                nc.vector.tensor_scalar_mul(
                                    at[:, :W], at[:, :W], r2[h2][:, qb:qb + 1]
                                )
                                h_ = hp * 2 + h2
                                nc.sync.dma_start(
                                    attn_o[h_, qsl, 0:W], at[:, :W]
                                )
                    if DEBUG and hp == 0:
                        nc.sync.dma_start(dbg_r2[:], r2[0][:])
                    # --- 2b: scoresT -> exp -> attn@v -> outT (per head) ---
                    for h2 in (0, 1):
                        base = h2 * 64
                        with (
                            tc.tile_pool(name="ps2b", bufs=2, space="PSUM") as ps2b,
                            tc.tile_pool(name="psO", bufs=1, space="PSUM") as psOp,
                            tc.tile_pool(name="ptr", bufs=2, space="PSUM") as ptrp,
                            tc.tile_pool(name="atT", bufs=3) as atT,
                            tc.tile_pool(name="bcp", bufs=2) as bcp,
                        ):
                            for c2 in range(2):
                                jmax = 8 * (c2 + 1)
                                isl0 = c2 * 1024
                                psO = psOp.tile([64, 1024], F32, tag="psO")
                                for jt in range(jmax):
                                    jsl = slice(jt * P, (jt + 1) * P)
                                    jd_lo = jt * P - isl0  # diag offset in chunk
                                    ih0 = max(0, jd_lo) // 512
                                    pre = max(0, jd_lo) - ih0 * 512
                                    ps2 = ps2b.tile([P, 1024], F32, tag="ps2")
                                    for ih in range(ih0, 2):
                                        nc.tensor.matmul(
                                            ps2[:, ih * 512:(ih + 1) * 512],
                                            kT_sb[base:base + 64, hp, jsl],
                                            qT_sb[base:base + 64, hp,
                                                  isl0 + ih * 512:
                                                  isl0 + (ih + 1) * 512],
                                            start=True, stop=True,
                                            tile_position=(base, 0),
                                        )
                                    if jd_lo >= 0:
                                        # mask j > i on the diagonal block
                                        nc.vector.tensor_tensor(
                                            ps2[:, jd_lo:jd_lo + P],
                                            ps2[:, jd_lo:jd_lo + P],
                                            maskT_sb[:], ADD,
                                        )
                                    aT = atT.tile([P, 1024], MM_DT, tag="aT")
                                    if pre > 0:
                                        nc.vector.memset(
                                            aT[:, ih0 * 512:ih0 * 512 + pre]
                                            .bitcast(F32),
                                            0.0,
                                        )
                                    nc.scalar.activation(
                                        aT[:, ih0 * 512 + pre:1024],
                                        ps2[:, ih0 * 512 + pre:1024], EXP_FN,
                                    )
                                    if DEBUG and hp == 0 and h2 == 0 and c2 == 0 and jt == 0:
                                        nc.sync.dma_start(dbg_aT[:], aT[:].bitcast(F32))
                                    vbase = (2 * hp + h2) * 64
                                    for ih in range(ih0, 2):
                                        last_jt = min(
                                            jmax - 1,
                                            (isl0 + (ih + 1) * 512) // P - 1,
                                        )
                                        nc.tensor.matmul(
                                            psO[:, ih * 512:(ih + 1) * 512],
                                            v_sb[:, jt, vbase:vbase + 64],
                                            aT[:, ih * 512:(ih + 1) * 512],
                                            start=(jt == 0), stop=(jt == last_jt),
                                        )
                                # broadcast rsum2 for these 8 q-blocks to [128,1024]
                                bc = bcp.tile([P, 1024], F32, tag="bc")
                                for qq in range(8):
                                    qbi = c2 * 8 + qq
                                    pt = ptrp.tile([1, P], F32, tag="pt")
                                    nc.tensor.transpose(
                                        pt[:], r2[h2][:, qbi:qbi + 1], ident[:]
                                    )
                                    row = stats.tile([1, P], F32, tag="row")
                                    nc.vector.tensor_copy(row[:], pt[:])
                                    nc.gpsimd.partition_broadcast(
                                        bc[:, qq * P:(qq + 1) * P], row[:]
                                    )
                                nc.vector.tensor_tensor(
                                    outT_sb[base:base + 64, hp,
                                            isl0:isl0 + 1024],
                                    psO[:], bc[0:64, :], MULT,
                                )
                                if DEBUG and hp == 0 and h2 == 0 and c2 == 0:
                                    nc.sync.dma_start(dbg_bc[:], bc[:])
                                    dps = bcp.tile([64, 1024], F32, tag="dps", name="dps")
                                    nc.vector.tensor_copy(dps[:], psO[:])
                                    nc.sync.dma_start(dbg_psO[:], dps[:])

            if DEBUG:
                with tc.tile_pool(name="dbgp", bufs=2) as dbgp:
                    for mt in range(4):
                        dt_ = dbgp.tile([P, S], F32, tag="dt")
                        nc.vector.tensor_copy(dt_[:], outT_sb[:, mt, :].bitcast(F32))
                        nc.sync.dma_start(dbg_outT[:, mt, :], dt_[:])
            # ---------------- Phase 3: out projection ----------------
            with (
                tc.tile_pool(name="w3", bufs=1) as w3,
                tc.tile_pool(name="ps3", bufs=4, space="PSUM") as ps3,
                tc.tile_pool(name="o3", bufs=3) as o3,
            ):
                wo_sb = w3.tile([P, 4, DM], MM_DT, tag="wo")
                nc.gpsimd.dma_start(wo_sb[:], wo_r)
                bo_sb = w3.tile([P, DM], F32, tag="bo")
                nc.sync.dma_start(bo_sb[:], bo[:])
                for st in range(NQB):
                    ssl = slice(st * P, (st + 1) * P)
                    for ncn in range(2):
                        nsl = slice(ncn * 512, (ncn + 1) * 512)
                        ps = ps3.tile([P, 512], F32, tag="ps")
                        for mt in range(4):
                            nc.tensor.matmul(
                                ps[:], outT_sb[:, mt, ssl], wo_sb[:, mt, nsl],
                                start=(mt == 0), stop=(mt == 3),
                            )
                        ot = o3.tile([P, 512], F32, tag="ot")
                        nc.vector.tensor_tensor(ot[:], ps[:], bo_sb[:, nsl], ADD)
                        nc.sync.dma_start(out_o[ssl, nsl], ot[:])

    nc.compile()
    return nc


_NC_CACHE = None
_last_in_maps = None


def _get_nc():
    global _NC_CACHE
    if _NC_CACHE is None:
        _NC_CACHE = _build()
    return _NC_CACHE


def _rope_tables():
    # mirror reference.apply_rope in float32
    inv_freq = (
        1.0 / (np.float32(10000.0)
               ** (np.arange(0, DH, 2, dtype=np.float32) / np.float32(DH)))
    ).astype(np.float32)
    t = np.arange(S, dtype=np.float32)
    freqs = t[:, None] * inv_freq[None, :]                    # [S, 32]
    cos = np.concatenate([np.cos(freqs), np.cos(freqs)], axis=-1)  # [S, 64]
    sin = np.concatenate([np.sin(freqs), np.sin(freqs)], axis=-1)
    cosT = np.ascontiguousarray(cos.T)                        # [64, S]
    sinT = np.ascontiguousarray(sin.T)
    cos2 = np.tile(cosT, (2, 1)).astype(np.float32)           # [128, S]
    sin2 = np.tile(sinT, (2, 1)).astype(np.float32)
    return cos2, sin2


def kernel(x, w_qkv, b_qkv, w_out, b_out):
    x = np.asarray(x, dtype=np.float32)
    w_qkv = np.asarray(w_qkv, dtype=np.float32)
    b_qkv = np.asarray(b_qkv, dtype=np.float32)
    w_out = np.asarray(w_out, dtype=np.float32)
    b_out = np.asarray(b_out, dtype=np.float32)

    nc = _get_nc()
    cos2, sin2 = _rope_tables()
    scale = np.float32(1.0 / np.sqrt(DH))  # 1/8, exact power of two
    cosq = cos2 * scale
    sinq = sin2 * scale
    mask = np.triu(np.full((P, P), -1e30, dtype=np.float32), 1)

    in_maps = []
    for c in range(8):
        b, g = c // 2, c % 2
        h0 = g * HL
        col = slice(h0 * DH, h0 * DH + ML)
        wq_s = np.ascontiguousarray(w_qkv[:, 0 * DM:1 * DM][:, col])
        wk_s = np.ascontiguousarray(w_qkv[:, 1 * DM:2 * DM][:, col])
        wv_s = np.ascontiguousarray(w_qkv[:, 2 * DM:3 * DM][:, col])
        bq_s = b_qkv[0 * DM:1 * DM][col].reshape(4, P).T
        bk_s = b_qkv[1 * DM:2 * DM][col].reshape(4, P).T
        bv_s = np.broadcast_to(b_qkv[2 * DM:3 * DM][col], (P, ML))
        wo_s = np.ascontiguousarray(w_out[g * ML:(g + 1) * ML, :])
        bo_eff = b_out if g == 0 else np.zeros_like(b_out)
        bo_s = np.broadcast_to(bo_eff, (P, DM))
        in_maps.append({
            "xT": np.ascontiguousarray(x[b].T),
            "wq": wq_s, "wk": wk_s, "wv": wv_s, "wo": wo_s,
            "bq": np.ascontiguousarray(bq_s),
            "bk": np.ascontiguousarray(bk_s),
            "bv": np.ascontiguousarray(bv_s),
            "bo": np.ascontiguousarray(bo_s),
            "cosq": cosq, "sinq": sinq, "cosk": cos2, "sink": sin2,
            "maskd": mask, "maskdT": np.ascontiguousarray(mask.T),
        })

    global _last_in_maps
    _last_in_maps = in_maps
    res = run_bass_kernel_spmd(nc, in_maps, core_ids=list(range(8)))

    out = np.empty((B, S, DM), dtype=np.float32)
    attn = np.empty((B, H, S, S), dtype=np.float32)
    for c in range(8):
        b, g = c // 2, c % 2
        r = res.results[c]
        attn[b, g * HL:(g + 1) * HL] = r["attn_o"]
        if g == 0:
            out[b] = r["out_o"]
        else:
            out[b] += r["out_o"]
    return out, attn
